# revision 40
# baseline (speedup 1.0000x reference)
"""GAT (2-layer, PyG-style) on 8 Trainium2 NeuronCores via Bass/Tile.

Strategy (dst-sharded, degree-sorted tiles) — v2:
- Nodes sharded by dst across 8 cores (12500 each). Per core, dsts are
  degree-sorted and grouped into 98 tiles of 128 (partition = dst).
- Per tile, column 0 gathers the dst's own table row (serving both the
  self-loop edge and the per-partition al_dst values); remaining columns
  hold in-edges, padded to the tile max degree with masked slots.
- Layer tables are 4-node-packed rows (<=32767 rows, int16 dma_gather
  indices); a 4-way one-hot select on DVE picks the node within the row.
- Node phase is sharded 8x: host passes x pre-transposed in bf16; each
  core computes 1/8 of table1 with PE matmuls (no on-chip transpose) and
  an AllGather assembles the full table.
- Edge phase: ONE dma_gather per <=48-column part (amortizes the ~3us
  fixed SWDGE call overhead), 4-queue rotation.
- LeakyReLU+exp with no DVE tensor_scalar: exp(lrelu(e)) =
  max(exp(e), exp(0.2*e)) via two scalar-engine Exp activations.
- elu via relu+exp identity: elu(y)+1 = relu(y) + exp(min(y,0)); the -1
  is folded into the layer-2 bias (b2e' = b2row - ones@w2e).
- Segment softmax: no max-subtraction needed (logits are O(1)); the
  denominator divides the aggregated numerator once per dst row.
- b1/b2 folded into the h-columns of the tables (alpha sums to 1).
- Layer-2 per-node features (4 values) are exchanged via an on-chip
  AllGather of 4-packed shards in core-local permuted order.
"""

import math

import numpy as np
import ml_dtypes

BF16 = ml_dtypes.bfloat16

N = 100_000
E = 3_200_000
IN = 128
H1, C1 = 8, 8
HID = H1 * C1          # 64
OUT = 2
NEG = 0.2
NCORES = 8
ND = N // NCORES       # dsts per core: 12500
NT = 98                # tiles per core (98*128 = 12544)
PT = NT * 128          # padded dst slots per core
NPAD = 100_352         # x padded to 784*128 (and divisible by 4 and 8)
SH = NPAD // NCORES    # node-phase shard: 12544 nodes per core
SHR = SH // 4          # 3136 4-pack rows per shard
T1ROWS = NPAD // 4     # 25088 4-pack rows, row = 4*80 vals pad to 384
T1W = 384              # bf16 elems per table1 row (768B)
T2ROWS = (PT * NCORES) // 4   # 25088
T2W = 128              # bf16 elems per table2 row (256B); 16 used
# Max columns per gather call / compute part, per layer. The SWDGE ring
# holds 1024 descriptors per queue and the gather ucode reserves a whole
# call up-front: descs ~= num_idxs * ceil(elem_bytes/256) / 16 + 1.
# L1 (768B rows): 32 cols = 4096 idx -> 769 descs. L2 (256B): 64 cols ok.
PARTC1 = 40
PARTC2 = 64
# Columns per dma_gather sub-call. Empirically 1024 indices per call is
# the hard limit regardless of element size (2048+ idx hangs or faults
# the device); descriptor-count formulas do NOT predict it.
GCOLS1 = 8
GCOLS2 = 8


def _wrap_idx(flat):
    """int16 index array -> [128, n/16] wrapped-in-16-partitions, replicated x8."""
    n = flat.shape[0]
    assert n % 16 == 0
    w = flat.reshape(n // 16, 16).T            # [16, n/16]
    return np.tile(w, (8, 1)).astype(np.int16)  # [128, n/16]


def _plan(src, dst):
    """Host-side index planning. Returns per-core data arrays + common schedule."""
    core = dst // ND
    dloc = dst % ND

    per_core = []
    for c in range(NCORES):
        m = core == c
        s_c = src[m]
        d_c = dloc[m]
        deg = np.bincount(d_c, minlength=ND)  # in-edges, no self loop yet
        order = np.argsort(-deg, kind="stable")  # degree-desc permutation
        perm = np.full(PT, -1, dtype=np.int64)
        perm[:ND] = order
        degp = np.zeros(PT, dtype=np.int64)
        degp[:ND] = deg[order]
        # group in-edges by dst for slot filling
        sort_by_d = np.argsort(d_c, kind="stable")
        s_sorted = s_c[sort_by_d]
        starts = np.zeros(ND + 1, dtype=np.int64)
        np.cumsum(deg, out=starts[1:])
        per_core.append(dict(perm=perm, degp=degp, s_sorted=s_sorted, starts=starts))

    # common K_t schedule: columns per tile = 1 (self/dst col) + max in-degree
    K = np.zeros(NT, dtype=np.int64)
    for t in range(NT):
        mx = 0
        for c in range(NCORES):
            d = per_core[c]["degp"][t * 128 : (t + 1) * 128]
            mx = max(mx, int(d.max()) if d.size else 0)
        K[t] = mx + 1
    ncols = int(K.sum())

    # balanced parts of <= PARTC columns; one gather call per part
    col0 = np.zeros(NT, dtype=np.int64)
    pos = 0
    for t in range(NT):
        col0[t] = pos
        pos += int(K[t])
    assert pos == ncols

    def make_parts(partc):
        parts = []   # (tile, gbase, kt, first, last)
        for t in range(NT):
            k = int(K[t])
            nparts = (k + partc - 1) // partc
            base = k // nparts
            rem = k % nparts
            off = 0
            for pi in range(nparts):
                kt = base + (1 if pi < rem else 0)
                parts.append((t, int(col0[t]) + off, kt, pi == 0,
                              pi == nparts - 1))
                off += kt
        return parts

    parts1 = make_parts(PARTC1)
    parts2 = make_parts(PARTC2)

    # per-core slot arrays
    datas = []
    for c in range(NCORES):
        pc = per_core[c]
        perm, degp, s_sorted, starts = (
            pc["perm"], pc["degp"], pc["s_sorted"], pc["starts"],
        )
        node1 = np.zeros((ncols, 128), dtype=np.int64)   # global node id (L1)
        valid = np.zeros((ncols, 128), dtype=bool)
        for t in range(NT):
            base = int(col0[t])
            d_orig = perm[t * 128 : (t + 1) * 128]           # local dst ids, -1 pad
            real = d_orig >= 0
            dg = np.where(real, d_orig, 0)
            # column 0: the dst's own row (self loop + al_dst source)
            node1[base, :] = c * ND + dg
            valid[base, :] = real
            # in-edge columns
            kt = int(K[t])
            if kt > 1:
                st = starts[dg]
                cnt = degp[t * 128 : (t + 1) * 128]
                for j in range(1, kt):
                    sel = (j - 1 < cnt) & real
                    idxs = st + (j - 1)
                    node1[base + j, sel] = s_sorted[np.where(sel, idxs, 0)][sel]
                    valid[base + j, sel] = True
        datas.append(dict(node1=node1, valid=valid, perm=pc["perm"]))
    return datas, K, col0, parts1, parts2, ncols


def _pack_inputs(datas, gpos_of_node, parts1, parts2, ncols):
    """Build per-core device input arrays from the slot plan."""
    per_core_inputs = []
    for c in range(NCORES):
        node1 = datas[c]["node1"]      # [ncols, 128]
        valid = datas[c]["valid"]

        idx1_flat = np.where(valid, node1 // 4, 0).astype(np.int16)
        ohm1 = np.zeros((ncols, 128, 5), dtype=BF16)
        ohv = np.eye(4, dtype=np.float32)[(node1 % 4)] * valid[:, :, None]
        ohm1[:, :, 0:4] = ohv.astype(BF16)
        ohm1[:, :, 4] = np.where(valid, 0.0, -1e30).astype(BF16)

        g = gpos_of_node[node1]        # permuted global position (L2 table)
        idx2_flat = np.where(valid, g // 4, 0).astype(np.int16)
        ohm2 = np.zeros((ncols, 128, 5), dtype=BF16)
        ohv2 = np.eye(4, dtype=np.float32)[(g % 4)] * valid[:, :, None]
        ohm2[:, :, 0:4] = ohv2.astype(BF16)
        ohm2[:, :, 4] = ohm1[:, :, 4]

        # wrap indices per gather part (layer-specific part boundaries)
        w1l = [_wrap_idx(idx1_flat[gb : gb + kt].reshape(-1))
               for (_t, gb, kt, _f, _l) in parts1]
        w2l = [_wrap_idx(idx2_flat[gb : gb + kt].reshape(-1))
               for (_t, gb, kt, _f, _l) in parts2]
        idx1_w = np.concatenate(w1l, axis=1)
        idx2_w = np.concatenate(w2l, axis=1)

        per_core_inputs.append(dict(
            idx1=idx1_w,
            idx2=idx2_w,
            ohm1=np.ascontiguousarray(ohm1.transpose(1, 0, 2)),
            ohm2=np.ascontiguousarray(ohm2.transpose(1, 0, 2)),
        ))
    return per_core_inputs


_BUILD_CACHE = {}


def _build(K, col0, parts1, parts2, totc, ncols):
    import concourse.bass as bass
    import concourse.bacc as bacc
    import concourse.mybir as mybir
    import concourse.tile as tile
    from concourse.masks import make_identity

    f32 = mybir.dt.float32
    bf16 = mybir.dt.bfloat16
    i16 = mybir.dt.int16
    AX = mybir.AxisListType.X
    OP = mybir.AluOpType
    ACT = mybir.ActivationFunctionType

    nc = bacc.Bacc("TRN2", target_bir_lowering=False, debug=False,
                   num_devices=NCORES, num_swdge_queues=4)

    xTh = nc.dram_tensor("xTh", [IN, SH], bf16, kind="ExternalInput")
    xTl = nc.dram_tensor("xTl", [IN, SH], bf16, kind="ExternalInput")
    w1eh = nc.dram_tensor("w1eh", [IN, 80], bf16, kind="ExternalInput")
    w1el = nc.dram_tensor("w1el", [IN, 80], bf16, kind="ExternalInput")
    b1e = nc.dram_tensor("b1e", [128, 80], f32, kind="ExternalInput")
    w2e = nc.dram_tensor("w2e", [HID, 4], bf16, kind="ExternalInput")
    b2e = nc.dram_tensor("b2e", [128, 4], f32, kind="ExternalInput")
    idx1 = nc.dram_tensor("idx1", [128, totc], i16, kind="ExternalInput")
    idx2 = nc.dram_tensor("idx2", [128, totc], i16, kind="ExternalInput")
    ohm1 = nc.dram_tensor("ohm1", [128, ncols, 5], bf16, kind="ExternalInput")
    ohm2 = nc.dram_tensor("ohm2", [128, ncols, 5], bf16, kind="ExternalInput")

    t1shard = nc.dram_tensor("t1shard", [SHR, T1W], bf16, kind="Internal")
    table1 = nc.dram_tensor("table1", [T1ROWS, T1W], bf16, kind="Internal",
                            addr_space="Shared")
    t2shard = nc.dram_tensor("t2shard", [PT // 4, T2W], bf16, kind="Internal")
    table2 = nc.dram_tensor("table2", [T2ROWS, T2W], bf16, kind="Internal",
                            addr_space="Shared")
    outp = nc.dram_tensor("outp", [PT, OUT], f32, kind="ExternalOutput")

    with tile.TileContext(nc) as tc:
        with (
            tc.tile_pool(name="const", bufs=1) as cpool,
            tc.tile_pool(name="node", bufs=3) as npool,
            tc.tile_pool(name="npsum", bufs=2, space="PSUM") as npsum,
            tc.tile_pool(name="gth", bufs=3) as gpool,
            tc.tile_pool(name="edge", bufs=3) as epool,
            tc.tile_pool(name="accs", bufs=2) as apool,
            tc.tile_pool(name="fin", bufs=2) as fpool,
            tc.tile_pool(name="fpsum", bufs=2, space="PSUM") as fpsum,
        ):
            ident = cpool.tile([128, 128], bf16)
            make_identity(nc, ident[:])
            zero1 = cpool.tile([128, 1], f32)
            nc.vector.memset(zero1[:], 0.0)
            w1hs = cpool.tile([IN, 80], bf16)
            nc.sync.dma_start(out=w1hs[:], in_=w1eh[:])
            w1ls = cpool.tile([IN, 80], bf16)
            nc.sync.dma_start(out=w1ls[:], in_=w1el[:])
            b1es = cpool.tile([128, 80], f32)
            nc.sync.dma_start(out=b1es[:], in_=b1e[:])
            w2es = cpool.tile([HID, 4], bf16)
            nc.sync.dma_start(out=w2es[:], in_=w2e[:])
            b2es = cpool.tile([128, 4], f32)
            nc.sync.dma_start(out=b2es[:], in_=b2e[:])

            # ---- node phase (sharded 8x): t1shard rows = [al_src | h+b1 | al_dst]
            CH = 512
            chunks = [(i * CH, CH) for i in range(SH // CH)]
            if SH % CH:
                chunks.append((SH - SH % CH, SH % CH))
            for (off, sz) in chunks:
                nb = sz // 128
                xhs = npool.tile([128, CH], bf16, tag="xhs")
                nc.sync.dma_start(out=xhs[:, 0:sz], in_=xTh[:, off : off + sz])
                xls = npool.tile([128, CH], bf16, tag="xls")
                nc.sync.dma_start(out=xls[:, 0:sz], in_=xTl[:, off : off + sz])
                ps = npsum.tile([128, 4, 80], f32, tag="ps")
                for i in range(nb):
                    # bf16x3: h = xh@Wh + xl@Wh + xh@Wl (~f32 accuracy)
                    nc.tensor.matmul(out=ps[:, i, :],
                                     lhsT=xhs[:, i * 128 : (i + 1) * 128],
                                     rhs=w1hs[:], start=True, stop=False)
                    nc.tensor.matmul(out=ps[:, i, :],
                                     lhsT=xls[:, i * 128 : (i + 1) * 128],
                                     rhs=w1hs[:], start=False, stop=False)
                    nc.tensor.matmul(out=ps[:, i, :],
                                     lhsT=xhs[:, i * 128 : (i + 1) * 128],
                                     rhs=w1ls[:], start=False, stop=True)
                t1c = npool.tile([128, 4, 80], bf16, tag="t1c")
                nc.vector.tensor_tensor(
                    out=t1c[:, 0:nb, :], in0=ps[:, 0:nb, :],
                    in1=b1es[:].unsqueeze(1).to_broadcast([128, nb, 80]),
                    op=OP.add)
                for i in range(nb):
                    r0 = off // 4 + 32 * i
                    dst_ap = t1shard[r0 : r0 + 32, 0:320].rearrange(
                        "r (n v) -> r n v", v=80)
                    nc.scalar.dma_start(out=dst_ap, in_=t1c[:, i, :])

            nc.gpsimd.collective_compute(
                "AllGather",
                OP.bypass,
                replica_groups=[list(range(NCORES))],
                ins=[t1shard[:]],
                outs=[table1[:]],
            )

            # ---- edge phase runner
            def select4(out_ap, gt, kt, voff, nv, ohm_t, tag, ew, ktmax):
                # 4-way one-hot select as copy + 3 predicated overwrites.
                # TensorCopy runs at 2-4x on DVE; the broadcast-mask
                # mult/add formulation ran at 1x (stride-0 operands
                # disable the 2x_1p mode). Pad slots (all-zero one-hot)
                # keep sub-node 0's finite values; the -1e30 pad mask
                # zeroes their exp weight downstream.
                nc.vector.tensor_copy(out=out_ap,
                                      in_=gt[:, 0:kt, voff : voff + nv])
                for i in range(1, 4):
                    # CopyPredicated wants an int mask; bf16 1.0 = 0x3F80
                    nc.vector.copy_predicated(
                        out=out_ap,
                        mask=ohm_t[:, 0:kt, i : i + 1].bitcast(i16)
                            .to_broadcast([128, kt, nv]),
                        data=gt[:, 0:kt, i * ew + voff : i * ew + voff + nv])

            def edge_phase(layer):
                if layer == 1:
                    idxT, ohmT, tabT, EW, NV, EWN = idx1, ohm1, table1, T1W, 72, 80
                    parts, KTMAX, GCOLS = parts1, PARTC1, GCOLS1
                else:
                    idxT, ohmT, tabT, EW, NV, EWN = idx2, ohm2, table2, T2W, 4, 4
                    parts, KTMAX, GCOLS = parts2, PARTC2, GCOLS2
                NH = H1 if layer == 1 else 1
                NCCH = C1 if layer == 1 else OUT
                TROWS = NH + NH * NCCH   # exp rows + weighted-payload rows

                ioff = 0
                gq = 0
                nalt = 0
                adt = None
                acc = None
                for (t, gbase, kt, first, last) in parts:
                    eng = nc.sync if (nalt % 2 == 0) else nc.scalar
                    nalt += 1
                    gt = gpool.tile([128, KTMAX, EW], bf16, tag=f"gt{layer}")
                    idx_t = epool.tile([128, KTMAX * 8], i16, tag=f"ix{layer}")
                    eng.dma_start(out=idx_t[:, 0 : kt * 8],
                                  in_=idxT[:, ioff : ioff + kt * 8])
                    for c0 in range(0, kt, GCOLS):
                        ncc = min(GCOLS, kt - c0)
                        nc.gpsimd.dma_gather(
                            gt[:, c0 : c0 + ncc, :], tabT[:],
                            idx_t[:, c0 * 8 : (c0 + ncc) * 8],
                            ncc * 128, ncc * 128, EW, queue_num=gq % 4)
                        gq += 1
                    ioff += kt * 8
                    ohm_t = epool.tile([128, KTMAX, 5], bf16, tag=f"oh{layer}")
                    eng.dma_start(out=ohm_t[:, 0:kt, :],
                                  in_=ohmT[:, gbase : gbase + kt, :])

                    V = epool.tile([128, KTMAX, NV], bf16, tag=f"V{layer}")
                    select4(V[:, 0:kt, :], gt, kt, 0, NV, ohm_t,
                            f"v{layer}", EWN, KTMAX)
                    if first:
                        if layer == 1:
                            adt_t = epool.tile([128, 1, NH], bf16,
                                               tag=f"adt{layer}")
                            select4(adt_t[:], gt, 1, NV, NH, ohm_t,
                                    f"a{layer}", EWN, KTMAX)
                            adt = adt_t[:]
                        else:
                            # L2 row = [a2s.g, g0, g1, a2d.g]; col 0 is the
                            # dst's own row, so al_dst is V[:, 0, 3]
                            adt = V[:, 0:1, 3:4]

                    eT = epool.tile([128, KTMAX, NH], f32, tag=f"e{layer}")
                    nc.vector.tensor_tensor(
                        out=eT[:, 0:kt, :], in0=V[:, 0:kt, 0:NH],
                        in1=adt.to_broadcast([128, kt, NH]),
                        op=OP.add)
                    nc.vector.tensor_tensor(
                        out=eT[:, 0:kt, :], in0=eT[:, 0:kt, :],
                        in1=ohm_t[:, 0:kt, 4:5].to_broadcast([128, kt, NH]),
                        op=OP.add)
                    # exp(lrelu(e)) = max(exp(e), exp(0.2e))
                    x1 = epool.tile([128, KTMAX, NH], bf16, tag=f"x1{layer}")
                    nc.scalar.activation(out=x1[:, 0:kt, :], in_=eT[:, 0:kt, :],
                                         func=ACT.Exp)
                    x2 = epool.tile([128, KTMAX, NH], bf16, tag=f"x2{layer}")
                    nc.scalar.activation(out=x2[:, 0:kt, :], in_=eT[:, 0:kt, :],
                                         func=ACT.Exp, scale=NEG)
                    W = epool.tile([128, TROWS, KTMAX], bf16,
                                   tag=f"W{layer}")
                    nc.vector.tensor_tensor(
                        out=W[:, 0:NH, 0:kt].rearrange("p h c -> p c h"),
                        in0=x1[:, 0:kt, :], in1=x2[:, 0:kt, :], op=OP.max)
                    nc.vector.tensor_tensor(
                        out=W[:, NH : NH + NH * NCCH, 0:kt].rearrange(
                            "p (h c) j -> p h c j", h=NH),
                        in0=V[:, 0:kt, NH : NH + NH * NCCH].rearrange(
                            "p j (h c) -> p h c j", h=NH),
                        in1=W[:, 0:NH, 0:kt].unsqueeze(2).to_broadcast(
                            [128, NH, NCCH, kt]),
                        op=OP.mult)
                    if first:
                        acc = apool.tile([128, TROWS], f32,
                                         tag=f"acc{layer}")
                        nc.vector.tensor_reduce(
                            out=acc[:], in_=W[:, :, 0:kt], axis=AX,
                            op=OP.add)
                    else:
                        red = apool.tile([128, TROWS], f32,
                                         tag=f"red{layer}")
                        nc.vector.tensor_reduce(
                            out=red[:], in_=W[:, :, 0:kt], axis=AX,
                            op=OP.add)
                        nc.vector.tensor_tensor(out=acc[:], in0=acc[:],
                                                in1=red[:], op=OP.add)
                    if last:
                        finalize(layer, t, acc)

            def finalize(layer, t, a):
                NH = H1 if layer == 1 else 1
                NCCH = C1 if layer == 1 else OUT
                rden = fpool.tile([128, NH], f32, tag="rden")
                nc.vector.reciprocal(out=rden[:], in_=a[:, 0:NH])
                if layer == 1:
                    y = fpool.tile([128, HID], f32, tag="y")
                    nc.vector.tensor_tensor(
                        out=y[:].rearrange("p (h c) -> p h c", h=NH),
                        in0=a[:, NH : NH + HID].rearrange("p (h c) -> p h c",
                                                          h=NH),
                        in1=rden[:].unsqueeze(2).to_broadcast([128, NH, NCCH]),
                        op=OP.mult)
                    # elu(y)+1 = relu(y) + exp(min(y,0)); -1 folded into b2e
                    rp = fpool.tile([128, HID], f32, tag="rp")
                    nc.vector.tensor_tensor(
                        out=rp[:], in0=y[:],
                        in1=zero1[:, 0:1].to_broadcast([128, HID]), op=OP.max)
                    zm = fpool.tile([128, HID], f32, tag="zm")
                    nc.vector.tensor_tensor(out=zm[:], in0=y[:], in1=rp[:],
                                            op=OP.subtract)
                    e1 = fpool.tile([128, HID], f32, tag="e1")
                    nc.scalar.activation(out=e1[:], in_=zm[:], func=ACT.Exp)
                    zb = fpool.tile([128, HID], bf16, tag="zb")
                    nc.vector.tensor_tensor(out=zb[:], in0=rp[:], in1=e1[:],
                                            op=OP.add)
                    # G2 row = zs @ W2e + b2e'
                    zT_ps = fpsum.tile([HID, 128], bf16, tag="zTp")
                    nc.tensor.transpose(out=zT_ps[:], in_=zb[:],
                                        identity=ident[:])
                    zTs = fpool.tile([HID, 128], bf16, tag="zTs")
                    nc.vector.tensor_copy(out=zTs[:], in_=zT_ps[:])
                    g2_ps = fpsum.tile([128, 4], f32, tag="g2p")
                    nc.tensor.matmul(out=g2_ps[:], lhsT=zTs[:], rhs=w2es[:],
                                     start=True, stop=True)
                    g2s = fpool.tile([128, 4], bf16, tag="g2s")
                    nc.vector.tensor_tensor(out=g2s[:], in0=g2_ps[:],
                                            in1=b2es[:], op=OP.add)
                    dst_ap = t2shard[t * 32 : t * 32 + 32, 0:16].rearrange(
                        "r (n v) -> r n v", v=4)
                    nc.sync.dma_start(out=dst_ap, in_=g2s[:])
                else:
                    o = fpool.tile([128, OUT], f32, tag="o2")
                    nc.vector.tensor_tensor(
                        out=o[:], in0=a[:, 1 : 1 + OUT],
                        in1=rden[:].to_broadcast([128, OUT]),
                        op=OP.mult)
                    nc.sync.dma_start(
                        out=outp[t * 128 : (t + 1) * 128, :], in_=o[:])

            edge_phase(1)

            nc.gpsimd.collective_compute(
                "AllGather",
                OP.bypass,
                replica_groups=[list(range(NCORES))],
                ins=[t2shard[:]],
                outs=[table2[:]],
            )

            edge_phase(2)

    nc.compile()
    return nc


def kernel(**inputs):
    from concourse.bass_utils import run_bass_kernel_spmd

    x = np.asarray(inputs["x"], dtype=np.float32)
    ei = np.asarray(inputs["edge_index"]).astype(np.int64)
    w1 = np.asarray(inputs["W1"], dtype=np.float32)
    a1s = np.asarray(inputs["a1_src"], dtype=np.float32)
    a1d = np.asarray(inputs["a1_dst"], dtype=np.float32)
    b1 = np.asarray(inputs["b1"], dtype=np.float32)
    w2 = np.asarray(inputs["W2"], dtype=np.float32)
    a2s = np.asarray(inputs["a2_src"], dtype=np.float32)
    a2d = np.asarray(inputs["a2_dst"], dtype=np.float32)
    b2 = np.asarray(inputs["b2"], dtype=np.float32)

    src = ei[0]
    dst = ei[1]

    datas, K, col0, parts1, parts2, ncols = _plan(src, dst)
    totc = 8 * ncols

    # permuted global position of each node for the L2 table
    gpos_of_node = np.zeros(NPAD, dtype=np.int64)
    for c in range(NCORES):
        perm = datas[c]["perm"]  # [PT] local dst ids (or -1)
        real = perm >= 0
        gpos_of_node[c * ND + perm[real]] = c * PT + np.nonzero(real)[0]

    per_core = _pack_inputs(datas, gpos_of_node, parts1, parts2, ncols)

    # weights
    A1s = np.zeros((HID, H1), dtype=np.float32)
    A1d = np.zeros((HID, H1), dtype=np.float32)
    for h in range(H1):
        A1s[h * C1 : (h + 1) * C1, h] = a1s[h]
        A1d[h * C1 : (h + 1) * C1, h] = a1d[h]
    w1e = np.concatenate([w1 @ A1s, w1, w1 @ A1d], axis=1)      # [128, 80]
    w2e = np.concatenate([w2 @ a2s.T, w2, w2 @ a2d.T], axis=1)  # [64, 4]
    b1e = np.zeros((128, 80), dtype=np.float32)
    b1e[:, 8 : 8 + HID] = b1[None, :]
    # b2 row with the elu -1 shift folded in: t2row = zs@w2e + (b2row - 1@w2e)
    b2row = np.zeros(4, dtype=np.float32)
    b2row[1 : 1 + OUT] = b2
    b2adj = b2row - np.ones(HID, dtype=np.float32) @ w2e
    b2e = np.tile(b2adj[None, :], (128, 1)).astype(np.float32)

    xp = np.zeros((NPAD, IN), dtype=np.float32)
    xp[:N] = x
    xT_f32 = xp.T                                      # [128, NPAD]
    xTh_all = xT_f32.astype(BF16)
    xTl_all = (xT_f32 - xTh_all.astype(np.float32)).astype(BF16)
    w1h = w1e.astype(BF16)
    w1l = (w1e - w1h.astype(np.float32)).astype(BF16)

    key = (PARTC1, PARTC2, tuple(K.tolist()))
    if key not in _BUILD_CACHE:
        _BUILD_CACHE[key] = _build(K, col0, parts1, parts2, totc, ncols)
    nc = _BUILD_CACHE[key]

    common = dict(w1eh=w1h, w1el=w1l, b1e=b1e, w2e=w2e.astype(BF16),
                  b2e=b2e)
    in_maps = []
    for c in range(NCORES):
        m = dict(common)
        m.update(per_core[c])
        m["xTh"] = np.ascontiguousarray(xTh_all[:, c * SH : (c + 1) * SH])
        m["xTl"] = np.ascontiguousarray(xTl_all[:, c * SH : (c + 1) * SH])
        in_maps.append(m)

    global _LAST_IN_MAPS
    _LAST_IN_MAPS = in_maps
    res = run_bass_kernel_spmd(nc, in_maps, list(range(NCORES)))

    out = np.zeros((N, OUT), dtype=np.float32)
    for c in range(NCORES):
        op = res.results[c]["outp"]       # [PT, 2] in permuted order
        perm = datas[c]["perm"]
        real = perm >= 0
        out[c * ND + perm[real]] = op[real]
    return out


# revision 41
# speedup vs baseline: 1.0243x; 1.0243x over previous
"""GAT (2-layer, PyG-style) on 8 Trainium2 NeuronCores via Bass/Tile.

Strategy (dst-sharded, degree-sorted tiles) — v2:
- Nodes sharded by dst across 8 cores (12500 each). Per core, dsts are
  degree-sorted and grouped into 98 tiles of 128 (partition = dst).
- Per tile, column 0 gathers the dst's own table row (serving both the
  self-loop edge and the per-partition al_dst values); remaining columns
  hold in-edges, padded to the tile max degree with masked slots.
- Layer tables are 4-node-packed rows (<=32767 rows, int16 dma_gather
  indices); a 4-way one-hot select on DVE picks the node within the row.
- Node phase is sharded 8x: host passes x pre-transposed in bf16; each
  core computes 1/8 of table1 with PE matmuls (no on-chip transpose) and
  an AllGather assembles the full table.
- Edge phase: ONE dma_gather per <=48-column part (amortizes the ~3us
  fixed SWDGE call overhead), 4-queue rotation.
- LeakyReLU+exp with no DVE tensor_scalar: exp(lrelu(e)) =
  max(exp(e), exp(0.2*e)) via two scalar-engine Exp activations.
- elu via relu+exp identity: elu(y)+1 = relu(y) + exp(min(y,0)); the -1
  is folded into the layer-2 bias (b2e' = b2row - ones@w2e).
- Segment softmax: no max-subtraction needed (logits are O(1)); the
  denominator divides the aggregated numerator once per dst row.
- b1/b2 folded into the h-columns of the tables (alpha sums to 1).
- Layer-2 per-node features (4 values) are exchanged via an on-chip
  AllGather of 4-packed shards in core-local permuted order.
"""

import math

import numpy as np
import ml_dtypes

BF16 = ml_dtypes.bfloat16

N = 100_000
E = 3_200_000
IN = 128
H1, C1 = 8, 8
HID = H1 * C1          # 64
OUT = 2
NEG = 0.2
NCORES = 8
ND = N // NCORES       # dsts per core: 12500
NT = 98                # tiles per core (98*128 = 12544)
PT = NT * 128          # padded dst slots per core
NPAD = 100_352         # x padded to 784*128 (and divisible by 4 and 8)
SH = NPAD // NCORES    # node-phase shard: 12544 nodes per core
SHR = SH // 4          # 3136 4-pack rows per shard
T1ROWS = NPAD // 4     # 25088 4-pack rows, row = 4*80 vals pad to 384
T1W = 384              # bf16 elems per table1 row (768B)
T2ROWS = (PT * NCORES) // 4   # 25088
T2W = 128              # bf16 elems per table2 row (256B); 16 used
# Max columns per gather call / compute part, per layer. The SWDGE ring
# holds 1024 descriptors per queue and the gather ucode reserves a whole
# call up-front: descs ~= num_idxs * ceil(elem_bytes/256) / 16 + 1.
# L1 (768B rows): 32 cols = 4096 idx -> 769 descs. L2 (256B): 64 cols ok.
PARTC1 = 40
PARTC2 = 64
# Columns per dma_gather sub-call. Empirically 8 cols (1024 idx) is the
# largest ring-safe call: 16-col 768B-row calls and 64-col 256B-row calls
# both deadlock the SWDGE ring on hardware.
GCOLS1 = 8
GCOLS2 = 8


def _wrap_idx(flat):
    """int16 index array -> [128, n/16] wrapped-in-16-partitions, replicated x8."""
    n = flat.shape[0]
    assert n % 16 == 0
    w = flat.reshape(n // 16, 16).T            # [16, n/16]
    return np.tile(w, (8, 1)).astype(np.int16)  # [128, n/16]


def _plan(src, dst):
    """Host-side index planning. Returns per-core data arrays + common schedule."""
    core = dst // ND
    dloc = dst % ND

    per_core = []
    for c in range(NCORES):
        m = core == c
        s_c = src[m]
        d_c = dloc[m]
        deg = np.bincount(d_c, minlength=ND)  # in-edges, no self loop yet
        order = np.argsort(-deg, kind="stable")  # degree-desc permutation
        perm = np.full(PT, -1, dtype=np.int64)
        perm[:ND] = order
        degp = np.zeros(PT, dtype=np.int64)
        degp[:ND] = deg[order]
        # group in-edges by dst for slot filling
        sort_by_d = np.argsort(d_c, kind="stable")
        s_sorted = s_c[sort_by_d]
        starts = np.zeros(ND + 1, dtype=np.int64)
        np.cumsum(deg, out=starts[1:])
        per_core.append(dict(perm=perm, degp=degp, s_sorted=s_sorted, starts=starts))

    # common K_t schedule: columns per tile = 1 (self/dst col) + max in-degree
    K = np.zeros(NT, dtype=np.int64)
    for t in range(NT):
        mx = 0
        for c in range(NCORES):
            d = per_core[c]["degp"][t * 128 : (t + 1) * 128]
            mx = max(mx, int(d.max()) if d.size else 0)
        K[t] = mx + 1
    ncols = int(K.sum())

    # balanced parts of <= PARTC columns; one gather call per part
    col0 = np.zeros(NT, dtype=np.int64)
    pos = 0
    for t in range(NT):
        col0[t] = pos
        pos += int(K[t])
    assert pos == ncols

    def make_parts(partc):
        parts = []   # (tile, gbase, kt, first, last)
        for t in range(NT):
            k = int(K[t])
            nparts = (k + partc - 1) // partc
            base = k // nparts
            rem = k % nparts
            off = 0
            for pi in range(nparts):
                kt = base + (1 if pi < rem else 0)
                parts.append((t, int(col0[t]) + off, kt, pi == 0,
                              pi == nparts - 1))
                off += kt
        return parts

    parts1 = make_parts(PARTC1)
    parts2 = make_parts(PARTC2)

    # per-core slot arrays
    datas = []
    for c in range(NCORES):
        pc = per_core[c]
        perm, degp, s_sorted, starts = (
            pc["perm"], pc["degp"], pc["s_sorted"], pc["starts"],
        )
        node1 = np.zeros((ncols, 128), dtype=np.int64)   # global node id (L1)
        valid = np.zeros((ncols, 128), dtype=bool)
        for t in range(NT):
            base = int(col0[t])
            d_orig = perm[t * 128 : (t + 1) * 128]           # local dst ids, -1 pad
            real = d_orig >= 0
            dg = np.where(real, d_orig, 0)
            # column 0: the dst's own row (self loop + al_dst source)
            node1[base, :] = c * ND + dg
            valid[base, :] = real
            # in-edge columns
            kt = int(K[t])
            if kt > 1:
                st = starts[dg]
                cnt = degp[t * 128 : (t + 1) * 128]
                for j in range(1, kt):
                    sel = (j - 1 < cnt) & real
                    idxs = st + (j - 1)
                    node1[base + j, sel] = s_sorted[np.where(sel, idxs, 0)][sel]
                    valid[base + j, sel] = True
        datas.append(dict(node1=node1, valid=valid, perm=pc["perm"]))
    return datas, K, col0, parts1, parts2, ncols


def _pack_inputs(datas, gpos_of_node, parts1, parts2, ncols):
    """Build per-core device input arrays from the slot plan."""
    per_core_inputs = []
    for c in range(NCORES):
        node1 = datas[c]["node1"]      # [ncols, 128]
        valid = datas[c]["valid"]

        idx1_flat = np.where(valid, node1 // 4, 0).astype(np.int16)
        ohm1 = np.zeros((ncols, 128, 5), dtype=BF16)
        ohv = np.eye(4, dtype=np.float32)[(node1 % 4)] * valid[:, :, None]
        ohm1[:, :, 0:4] = ohv.astype(BF16)
        ohm1[:, :, 4] = np.where(valid, 0.0, -1e30).astype(BF16)

        g = gpos_of_node[node1]        # permuted global position (L2 table)
        idx2_flat = np.where(valid, g // 4, 0).astype(np.int16)
        ohm2 = np.zeros((ncols, 128, 5), dtype=BF16)
        ohv2 = np.eye(4, dtype=np.float32)[(g % 4)] * valid[:, :, None]
        ohm2[:, :, 0:4] = ohv2.astype(BF16)
        ohm2[:, :, 4] = ohm1[:, :, 4]

        # wrap indices per gather part (layer-specific part boundaries)
        w1l = [_wrap_idx(idx1_flat[gb : gb + kt].reshape(-1))
               for (_t, gb, kt, _f, _l) in parts1]
        w2l = [_wrap_idx(idx2_flat[gb : gb + kt].reshape(-1))
               for (_t, gb, kt, _f, _l) in parts2]
        idx1_w = np.concatenate(w1l, axis=1)
        idx2_w = np.concatenate(w2l, axis=1)

        per_core_inputs.append(dict(
            idx1=idx1_w,
            idx2=idx2_w,
            ohm1=np.ascontiguousarray(ohm1.transpose(1, 0, 2)),
            ohm2=np.ascontiguousarray(ohm2.transpose(1, 0, 2)),
        ))
    return per_core_inputs


_BUILD_CACHE = {}


def _build(K, col0, parts1, parts2, totc, ncols):
    import concourse.bass as bass
    import concourse.bacc as bacc
    import concourse.mybir as mybir
    import concourse.tile as tile
    from concourse.masks import make_identity

    f32 = mybir.dt.float32
    bf16 = mybir.dt.bfloat16
    i16 = mybir.dt.int16
    AX = mybir.AxisListType.X
    OP = mybir.AluOpType
    ACT = mybir.ActivationFunctionType

    nc = bacc.Bacc("TRN2", target_bir_lowering=False, debug=False,
                   num_devices=NCORES, num_swdge_queues=4)

    xTh = nc.dram_tensor("xTh", [IN, SH], bf16, kind="ExternalInput")
    xTl = nc.dram_tensor("xTl", [IN, SH], bf16, kind="ExternalInput")
    w1eh = nc.dram_tensor("w1eh", [IN, 80], bf16, kind="ExternalInput")
    w1el = nc.dram_tensor("w1el", [IN, 80], bf16, kind="ExternalInput")
    b1e = nc.dram_tensor("b1e", [128, 80], f32, kind="ExternalInput")
    w2e = nc.dram_tensor("w2e", [HID, 4], bf16, kind="ExternalInput")
    b2e = nc.dram_tensor("b2e", [128, 4], f32, kind="ExternalInput")
    idx1 = nc.dram_tensor("idx1", [128, totc], i16, kind="ExternalInput")
    idx2 = nc.dram_tensor("idx2", [128, totc], i16, kind="ExternalInput")
    ohm1 = nc.dram_tensor("ohm1", [128, ncols, 5], bf16, kind="ExternalInput")
    ohm2 = nc.dram_tensor("ohm2", [128, ncols, 5], bf16, kind="ExternalInput")

    t1shard = nc.dram_tensor("t1shard", [SHR, T1W], bf16, kind="Internal")
    table1 = nc.dram_tensor("table1", [T1ROWS, T1W], bf16, kind="Internal",
                            addr_space="Shared")
    t2shard = nc.dram_tensor("t2shard", [PT // 4, T2W], bf16, kind="Internal")
    table2 = nc.dram_tensor("table2", [T2ROWS, T2W], bf16, kind="Internal",
                            addr_space="Shared")
    outp = nc.dram_tensor("outp", [PT, OUT], f32, kind="ExternalOutput")

    with tile.TileContext(nc) as tc:
        with (
            tc.tile_pool(name="const", bufs=1) as cpool,
            tc.tile_pool(name="node", bufs=3) as npool,
            tc.tile_pool(name="npsum", bufs=2, space="PSUM") as npsum,
            tc.tile_pool(name="gth", bufs=2) as gpool,
            tc.tile_pool(name="edge", bufs=3) as epool,
            tc.tile_pool(name="accs", bufs=2) as apool,
            tc.tile_pool(name="fin", bufs=2) as fpool,
            tc.tile_pool(name="fpsum", bufs=2, space="PSUM") as fpsum,
        ):
            ident = cpool.tile([128, 128], bf16)
            make_identity(nc, ident[:])
            zero1 = cpool.tile([128, 1], f32)
            nc.vector.memset(zero1[:], 0.0)
            w1hs = cpool.tile([IN, 80], bf16)
            nc.sync.dma_start(out=w1hs[:], in_=w1eh[:])
            w1ls = cpool.tile([IN, 80], bf16)
            nc.sync.dma_start(out=w1ls[:], in_=w1el[:])
            b1es = cpool.tile([128, 80], f32)
            nc.sync.dma_start(out=b1es[:], in_=b1e[:])
            w2es = cpool.tile([HID, 4], bf16)
            nc.sync.dma_start(out=w2es[:], in_=w2e[:])
            b2es = cpool.tile([128, 4], f32)
            nc.sync.dma_start(out=b2es[:], in_=b2e[:])

            # ---- node phase (sharded 8x): t1shard rows = [al_src | h+b1 | al_dst]
            CH = 512
            chunks = [(i * CH, CH) for i in range(SH // CH)]
            if SH % CH:
                chunks.append((SH - SH % CH, SH % CH))
            for (off, sz) in chunks:
                nb = sz // 128
                xhs = npool.tile([128, CH], bf16, tag="xhs")
                nc.sync.dma_start(out=xhs[:, 0:sz], in_=xTh[:, off : off + sz])
                xls = npool.tile([128, CH], bf16, tag="xls")
                nc.sync.dma_start(out=xls[:, 0:sz], in_=xTl[:, off : off + sz])
                ps = npsum.tile([128, 4, 80], f32, tag="ps")
                for i in range(nb):
                    # bf16x3: h = xh@Wh + xl@Wh + xh@Wl (~f32 accuracy)
                    nc.tensor.matmul(out=ps[:, i, :],
                                     lhsT=xhs[:, i * 128 : (i + 1) * 128],
                                     rhs=w1hs[:], start=True, stop=False)
                    nc.tensor.matmul(out=ps[:, i, :],
                                     lhsT=xls[:, i * 128 : (i + 1) * 128],
                                     rhs=w1hs[:], start=False, stop=False)
                    nc.tensor.matmul(out=ps[:, i, :],
                                     lhsT=xhs[:, i * 128 : (i + 1) * 128],
                                     rhs=w1ls[:], start=False, stop=True)
                t1c = npool.tile([128, 4, 80], bf16, tag="t1c")
                nc.vector.tensor_tensor(
                    out=t1c[:, 0:nb, :], in0=ps[:, 0:nb, :],
                    in1=b1es[:].unsqueeze(1).to_broadcast([128, nb, 80]),
                    op=OP.add)
                for i in range(nb):
                    r0 = off // 4 + 32 * i
                    dst_ap = t1shard[r0 : r0 + 32, 0:320].rearrange(
                        "r (n v) -> r n v", v=80)
                    nc.scalar.dma_start(out=dst_ap, in_=t1c[:, i, :])

            nc.gpsimd.collective_compute(
                "AllGather",
                OP.bypass,
                replica_groups=[list(range(NCORES))],
                ins=[t1shard[:]],
                outs=[table1[:]],
            )

            # ---- edge phase runner
            def select4(out_ap, gt, kt, voff, nv, ohm_t, tag, ew, ktmax):
                # 4-way one-hot select as copy + 3 predicated overwrites.
                # TensorCopy runs at 2-4x on DVE; the broadcast-mask
                # mult/add formulation ran at 1x (stride-0 operands
                # disable the 2x_1p mode). Pad slots (all-zero one-hot)
                # keep sub-node 0's finite values; the -1e30 pad mask
                # zeroes their exp weight downstream.
                nc.vector.tensor_copy(out=out_ap,
                                      in_=gt[:, 0:kt, voff : voff + nv])
                for i in range(1, 4):
                    # CopyPredicated wants an int mask; bf16 1.0 = 0x3F80
                    nc.vector.copy_predicated(
                        out=out_ap,
                        mask=ohm_t[:, 0:kt, i : i + 1].bitcast(i16)
                            .to_broadcast([128, kt, nv]),
                        data=gt[:, 0:kt, i * ew + voff : i * ew + voff + nv])

            def edge_phase(layer):
                if layer == 1:
                    idxT, ohmT, tabT, EW, NV, EWN = idx1, ohm1, table1, T1W, 72, 80
                    parts, KTMAX, GCOLS = parts1, PARTC1, GCOLS1
                else:
                    idxT, ohmT, tabT, EW, NV, EWN = idx2, ohm2, table2, T2W, 4, 4
                    parts, KTMAX, GCOLS = parts2, PARTC2, GCOLS2
                NH = H1 if layer == 1 else 1
                NCCH = C1 if layer == 1 else OUT
                TROWS = NH + NH * NCCH   # exp rows + weighted-payload rows

                ioff = 0
                gq = 0
                nalt = 0
                adt = None
                acc = None
                for (t, gbase, kt, first, last) in parts:
                    eng = nc.sync if (nalt % 2 == 0) else nc.scalar
                    nalt += 1
                    gt = gpool.tile([128, KTMAX, EW], bf16, tag=f"gt{layer}")
                    idx_t = epool.tile([128, KTMAX * 8], i16, tag=f"ix{layer}")
                    eng.dma_start(out=idx_t[:, 0 : kt * 8],
                                  in_=idxT[:, ioff : ioff + kt * 8])
                    for c0 in range(0, kt, GCOLS):
                        ncc = min(GCOLS, kt - c0)
                        nc.gpsimd.dma_gather(
                            gt[:, c0 : c0 + ncc, :], tabT[:],
                            idx_t[:, c0 * 8 : (c0 + ncc) * 8],
                            ncc * 128, ncc * 128, EW, queue_num=gq % 4)
                        gq += 1
                    ioff += kt * 8
                    ohm_t = epool.tile([128, KTMAX, 5], bf16, tag=f"oh{layer}")
                    eng.dma_start(out=ohm_t[:, 0:kt, :],
                                  in_=ohmT[:, gbase : gbase + kt, :])

                    V = epool.tile([128, KTMAX, NV], bf16, tag=f"V{layer}")
                    select4(V[:, 0:kt, :], gt, kt, 0, NV, ohm_t,
                            f"v{layer}", EWN, KTMAX)
                    if first:
                        if layer == 1:
                            adt_t = epool.tile([128, 1, NH], bf16,
                                               tag=f"adt{layer}")
                            select4(adt_t[:], gt, 1, NV, NH, ohm_t,
                                    f"a{layer}", EWN, KTMAX)
                            adt = adt_t[:]
                        else:
                            # L2 row = [a2s.g, g0, g1, a2d.g]; col 0 is the
                            # dst's own row, so al_dst is V[:, 0, 3]
                            adt = V[:, 0:1, 3:4]

                    eT = epool.tile([128, KTMAX, NH], f32, tag=f"e{layer}")
                    nc.vector.tensor_tensor(
                        out=eT[:, 0:kt, :], in0=V[:, 0:kt, 0:NH],
                        in1=adt.to_broadcast([128, kt, NH]),
                        op=OP.add)
                    nc.vector.tensor_tensor(
                        out=eT[:, 0:kt, :], in0=eT[:, 0:kt, :],
                        in1=ohm_t[:, 0:kt, 4:5].to_broadcast([128, kt, NH]),
                        op=OP.add)
                    # exp(lrelu(e)) = max(exp(e), exp(0.2e))
                    x1 = epool.tile([128, KTMAX, NH], bf16, tag=f"x1{layer}")
                    nc.scalar.activation(out=x1[:, 0:kt, :], in_=eT[:, 0:kt, :],
                                         func=ACT.Exp)
                    x2 = epool.tile([128, KTMAX, NH], bf16, tag=f"x2{layer}")
                    nc.scalar.activation(out=x2[:, 0:kt, :], in_=eT[:, 0:kt, :],
                                         func=ACT.Exp, scale=NEG)
                    W = epool.tile([128, TROWS, KTMAX], bf16,
                                   tag=f"W{layer}")
                    nc.vector.tensor_tensor(
                        out=W[:, 0:NH, 0:kt].rearrange("p h c -> p c h"),
                        in0=x1[:, 0:kt, :], in1=x2[:, 0:kt, :], op=OP.max)
                    nc.vector.tensor_tensor(
                        out=W[:, NH : NH + NH * NCCH, 0:kt].rearrange(
                            "p (h c) j -> p h c j", h=NH),
                        in0=V[:, 0:kt, NH : NH + NH * NCCH].rearrange(
                            "p j (h c) -> p h c j", h=NH),
                        in1=W[:, 0:NH, 0:kt].unsqueeze(2).to_broadcast(
                            [128, NH, NCCH, kt]),
                        op=OP.mult)
                    if first:
                        acc = apool.tile([128, TROWS], f32,
                                         tag=f"acc{layer}")
                        nc.vector.tensor_reduce(
                            out=acc[:], in_=W[:, :, 0:kt], axis=AX,
                            op=OP.add)
                    else:
                        red = apool.tile([128, TROWS], f32,
                                         tag=f"red{layer}")
                        nc.vector.tensor_reduce(
                            out=red[:], in_=W[:, :, 0:kt], axis=AX,
                            op=OP.add)
                        nc.vector.tensor_tensor(out=acc[:], in0=acc[:],
                                                in1=red[:], op=OP.add)
                    if last:
                        finalize(layer, t, acc)

            def finalize(layer, t, a):
                NH = H1 if layer == 1 else 1
                NCCH = C1 if layer == 1 else OUT
                rden = fpool.tile([128, NH], f32, tag="rden")
                nc.vector.reciprocal(out=rden[:], in_=a[:, 0:NH])
                if layer == 1:
                    y = fpool.tile([128, HID], f32, tag="y")
                    nc.vector.tensor_tensor(
                        out=y[:].rearrange("p (h c) -> p h c", h=NH),
                        in0=a[:, NH : NH + HID].rearrange("p (h c) -> p h c",
                                                          h=NH),
                        in1=rden[:].unsqueeze(2).to_broadcast([128, NH, NCCH]),
                        op=OP.mult)
                    # elu(y)+1 = relu(y) + exp(min(y,0)); -1 folded into b2e
                    rp = fpool.tile([128, HID], f32, tag="rp")
                    nc.vector.tensor_tensor(
                        out=rp[:], in0=y[:],
                        in1=zero1[:, 0:1].to_broadcast([128, HID]), op=OP.max)
                    zm = fpool.tile([128, HID], f32, tag="zm")
                    nc.vector.tensor_tensor(out=zm[:], in0=y[:], in1=rp[:],
                                            op=OP.subtract)
                    e1 = fpool.tile([128, HID], f32, tag="e1")
                    nc.scalar.activation(out=e1[:], in_=zm[:], func=ACT.Exp)
                    zb = fpool.tile([128, HID], bf16, tag="zb")
                    nc.vector.tensor_tensor(out=zb[:], in0=rp[:], in1=e1[:],
                                            op=OP.add)
                    # G2 row = zs @ W2e + b2e'
                    zT_ps = fpsum.tile([HID, 128], bf16, tag="zTp")
                    nc.tensor.transpose(out=zT_ps[:], in_=zb[:],
                                        identity=ident[:])
                    zTs = fpool.tile([HID, 128], bf16, tag="zTs")
                    nc.vector.tensor_copy(out=zTs[:], in_=zT_ps[:])
                    g2_ps = fpsum.tile([128, 4], f32, tag="g2p")
                    nc.tensor.matmul(out=g2_ps[:], lhsT=zTs[:], rhs=w2es[:],
                                     start=True, stop=True)
                    g2s = fpool.tile([128, 4], bf16, tag="g2s")
                    nc.vector.tensor_tensor(out=g2s[:], in0=g2_ps[:],
                                            in1=b2es[:], op=OP.add)
                    dst_ap = t2shard[t * 32 : t * 32 + 32, 0:16].rearrange(
                        "r (n v) -> r n v", v=4)
                    nc.sync.dma_start(out=dst_ap, in_=g2s[:])
                else:
                    o = fpool.tile([128, OUT], f32, tag="o2")
                    nc.vector.tensor_tensor(
                        out=o[:], in0=a[:, 1 : 1 + OUT],
                        in1=rden[:].to_broadcast([128, OUT]),
                        op=OP.mult)
                    nc.sync.dma_start(
                        out=outp[t * 128 : (t + 1) * 128, :], in_=o[:])

            edge_phase(1)

            nc.gpsimd.collective_compute(
                "AllGather",
                OP.bypass,
                replica_groups=[list(range(NCORES))],
                ins=[t2shard[:]],
                outs=[table2[:]],
            )

            edge_phase(2)

    nc.compile()
    return nc


def kernel(**inputs):
    from concourse.bass_utils import run_bass_kernel_spmd

    x = np.asarray(inputs["x"], dtype=np.float32)
    ei = np.asarray(inputs["edge_index"]).astype(np.int64)
    w1 = np.asarray(inputs["W1"], dtype=np.float32)
    a1s = np.asarray(inputs["a1_src"], dtype=np.float32)
    a1d = np.asarray(inputs["a1_dst"], dtype=np.float32)
    b1 = np.asarray(inputs["b1"], dtype=np.float32)
    w2 = np.asarray(inputs["W2"], dtype=np.float32)
    a2s = np.asarray(inputs["a2_src"], dtype=np.float32)
    a2d = np.asarray(inputs["a2_dst"], dtype=np.float32)
    b2 = np.asarray(inputs["b2"], dtype=np.float32)

    src = ei[0]
    dst = ei[1]

    datas, K, col0, parts1, parts2, ncols = _plan(src, dst)
    totc = 8 * ncols

    # permuted global position of each node for the L2 table
    gpos_of_node = np.zeros(NPAD, dtype=np.int64)
    for c in range(NCORES):
        perm = datas[c]["perm"]  # [PT] local dst ids (or -1)
        real = perm >= 0
        gpos_of_node[c * ND + perm[real]] = c * PT + np.nonzero(real)[0]

    per_core = _pack_inputs(datas, gpos_of_node, parts1, parts2, ncols)

    # weights
    A1s = np.zeros((HID, H1), dtype=np.float32)
    A1d = np.zeros((HID, H1), dtype=np.float32)
    for h in range(H1):
        A1s[h * C1 : (h + 1) * C1, h] = a1s[h]
        A1d[h * C1 : (h + 1) * C1, h] = a1d[h]
    w1e = np.concatenate([w1 @ A1s, w1, w1 @ A1d], axis=1)      # [128, 80]
    w2e = np.concatenate([w2 @ a2s.T, w2, w2 @ a2d.T], axis=1)  # [64, 4]
    b1e = np.zeros((128, 80), dtype=np.float32)
    b1e[:, 8 : 8 + HID] = b1[None, :]
    # b2 row with the elu -1 shift folded in: t2row = zs@w2e + (b2row - 1@w2e)
    b2row = np.zeros(4, dtype=np.float32)
    b2row[1 : 1 + OUT] = b2
    b2adj = b2row - np.ones(HID, dtype=np.float32) @ w2e
    b2e = np.tile(b2adj[None, :], (128, 1)).astype(np.float32)

    xp = np.zeros((NPAD, IN), dtype=np.float32)
    xp[:N] = x
    xT_f32 = xp.T                                      # [128, NPAD]
    xTh_all = xT_f32.astype(BF16)
    xTl_all = (xT_f32 - xTh_all.astype(np.float32)).astype(BF16)
    w1h = w1e.astype(BF16)
    w1l = (w1e - w1h.astype(np.float32)).astype(BF16)

    key = (PARTC1, PARTC2, tuple(K.tolist()))
    if key not in _BUILD_CACHE:
        _BUILD_CACHE[key] = _build(K, col0, parts1, parts2, totc, ncols)
    nc = _BUILD_CACHE[key]

    common = dict(w1eh=w1h, w1el=w1l, b1e=b1e, w2e=w2e.astype(BF16),
                  b2e=b2e)
    in_maps = []
    for c in range(NCORES):
        m = dict(common)
        m.update(per_core[c])
        m["xTh"] = np.ascontiguousarray(xTh_all[:, c * SH : (c + 1) * SH])
        m["xTl"] = np.ascontiguousarray(xTl_all[:, c * SH : (c + 1) * SH])
        in_maps.append(m)

    global _LAST_IN_MAPS
    _LAST_IN_MAPS = in_maps
    res = run_bass_kernel_spmd(nc, in_maps, list(range(NCORES)))

    out = np.zeros((N, OUT), dtype=np.float32)
    for c in range(NCORES):
        op = res.results[c]["outp"]       # [PT, 2] in permuted order
        perm = datas[c]["perm"]
        real = perm >= 0
        out[c * ND + perm[real]] = op[real]
    return out


# revision 42
# speedup vs baseline: 1.0462x; 1.0214x over previous
"""GAT (2-layer, PyG-style) on 8 Trainium2 NeuronCores via Bass/Tile.

Strategy (dst-sharded, degree-sorted tiles) — v2:
- Nodes sharded by dst across 8 cores (12500 each). Per core, dsts are
  degree-sorted and grouped into 98 tiles of 128 (partition = dst).
- Per tile, column 0 gathers the dst's own table row (serving both the
  self-loop edge and the per-partition al_dst values); remaining columns
  hold in-edges, padded to the tile max degree with masked slots.
- Layer tables are 4-node-packed rows (<=32767 rows, int16 dma_gather
  indices); a 4-way one-hot select on DVE picks the node within the row.
- Node phase is sharded 8x: host passes x pre-transposed in bf16; each
  core computes 1/8 of table1 with PE matmuls (no on-chip transpose) and
  an AllGather assembles the full table.
- Edge phase: ONE dma_gather per <=48-column part (amortizes the ~3us
  fixed SWDGE call overhead), 4-queue rotation.
- LeakyReLU+exp with no DVE tensor_scalar: exp(lrelu(e)) =
  max(exp(e), exp(0.2*e)) via two scalar-engine Exp activations.
- elu via relu+exp identity: elu(y)+1 = relu(y) + exp(min(y,0)); the -1
  is folded into the layer-2 bias (b2e' = b2row - ones@w2e).
- Segment softmax: no max-subtraction needed (logits are O(1)); the
  denominator divides the aggregated numerator once per dst row.
- b1/b2 folded into the h-columns of the tables (alpha sums to 1).
- Layer-2 per-node features (4 values) are exchanged via an on-chip
  AllGather of 4-packed shards in core-local permuted order.
"""

import math

import numpy as np
import ml_dtypes

BF16 = ml_dtypes.bfloat16

N = 100_000
E = 3_200_000
IN = 128
H1, C1 = 8, 8
HID = H1 * C1          # 64
OUT = 2
NEG = 0.2
NCORES = 8
ND = N // NCORES       # dsts per core: 12500
NT = 98                # tiles per core (98*128 = 12544)
PT = NT * 128          # padded dst slots per core
NPAD = 100_352         # x padded to 784*128 (and divisible by 4 and 8)
SH = NPAD // NCORES    # node-phase shard: 12544 nodes per core
SHR = SH // 4          # 3136 4-pack rows per shard
T1ROWS = NPAD // 4     # 25088 4-pack rows, row = 4*80 vals pad to 384
T1W = 384              # bf16 elems per table1 row (768B)
T2ROWS = (PT * NCORES) // 4   # 25088
T2W = 128              # bf16 elems per table2 row (256B); 16 used
# Max columns per gather call / compute part, per layer. The SWDGE ring
# holds 1024 descriptors per queue and the gather ucode reserves a whole
# call up-front: descs ~= num_idxs * ceil(elem_bytes/256) / 16 + 1.
# L1 (768B rows): 32 cols = 4096 idx -> 769 descs. L2 (256B): 64 cols ok.
PARTC1 = 48
PARTC2 = 64
# Columns per dma_gather sub-call. Empirically 8 cols (1024 idx) is the
# largest ring-safe call: 16-col 768B-row calls and 64-col 256B-row calls
# both deadlock the SWDGE ring on hardware.
GCOLS1 = 8
GCOLS2 = 8


def _wrap_idx(flat):
    """int16 index array -> [128, n/16] wrapped-in-16-partitions, replicated x8."""
    n = flat.shape[0]
    assert n % 16 == 0
    w = flat.reshape(n // 16, 16).T            # [16, n/16]
    return np.tile(w, (8, 1)).astype(np.int16)  # [128, n/16]


def _plan(src, dst):
    """Host-side index planning. Returns per-core data arrays + common schedule."""
    core = dst // ND
    dloc = dst % ND

    per_core = []
    for c in range(NCORES):
        m = core == c
        s_c = src[m]
        d_c = dloc[m]
        deg = np.bincount(d_c, minlength=ND)  # in-edges, no self loop yet
        order = np.argsort(-deg, kind="stable")  # degree-desc permutation
        perm = np.full(PT, -1, dtype=np.int64)
        perm[:ND] = order
        degp = np.zeros(PT, dtype=np.int64)
        degp[:ND] = deg[order]
        # group in-edges by dst for slot filling
        sort_by_d = np.argsort(d_c, kind="stable")
        s_sorted = s_c[sort_by_d]
        starts = np.zeros(ND + 1, dtype=np.int64)
        np.cumsum(deg, out=starts[1:])
        per_core.append(dict(perm=perm, degp=degp, s_sorted=s_sorted, starts=starts))

    # common K_t schedule: columns per tile = 1 (self/dst col) + max in-degree
    K = np.zeros(NT, dtype=np.int64)
    for t in range(NT):
        mx = 0
        for c in range(NCORES):
            d = per_core[c]["degp"][t * 128 : (t + 1) * 128]
            mx = max(mx, int(d.max()) if d.size else 0)
        K[t] = mx + 1
    ncols = int(K.sum())

    # balanced parts of <= PARTC columns; one gather call per part
    col0 = np.zeros(NT, dtype=np.int64)
    pos = 0
    for t in range(NT):
        col0[t] = pos
        pos += int(K[t])
    assert pos == ncols

    def make_parts(partc):
        parts = []   # (tile, gbase, kt, first, last)
        for t in range(NT):
            k = int(K[t])
            nparts = (k + partc - 1) // partc
            base = k // nparts
            rem = k % nparts
            off = 0
            for pi in range(nparts):
                kt = base + (1 if pi < rem else 0)
                parts.append((t, int(col0[t]) + off, kt, pi == 0,
                              pi == nparts - 1))
                off += kt
        return parts

    parts1 = make_parts(PARTC1)
    parts2 = make_parts(PARTC2)

    # per-core slot arrays
    datas = []
    for c in range(NCORES):
        pc = per_core[c]
        perm, degp, s_sorted, starts = (
            pc["perm"], pc["degp"], pc["s_sorted"], pc["starts"],
        )
        node1 = np.zeros((ncols, 128), dtype=np.int64)   # global node id (L1)
        valid = np.zeros((ncols, 128), dtype=bool)
        for t in range(NT):
            base = int(col0[t])
            d_orig = perm[t * 128 : (t + 1) * 128]           # local dst ids, -1 pad
            real = d_orig >= 0
            dg = np.where(real, d_orig, 0)
            # column 0: the dst's own row (self loop + al_dst source)
            node1[base, :] = c * ND + dg
            valid[base, :] = real
            # in-edge columns
            kt = int(K[t])
            if kt > 1:
                st = starts[dg]
                cnt = degp[t * 128 : (t + 1) * 128]
                for j in range(1, kt):
                    sel = (j - 1 < cnt) & real
                    idxs = st + (j - 1)
                    node1[base + j, sel] = s_sorted[np.where(sel, idxs, 0)][sel]
                    valid[base + j, sel] = True
        datas.append(dict(node1=node1, valid=valid, perm=pc["perm"]))
    return datas, K, col0, parts1, parts2, ncols


def _pack_inputs(datas, gpos_of_node, parts1, parts2, ncols):
    """Build per-core device input arrays from the slot plan."""
    per_core_inputs = []
    for c in range(NCORES):
        node1 = datas[c]["node1"]      # [ncols, 128]
        valid = datas[c]["valid"]

        idx1_flat = np.where(valid, node1 // 4, 0).astype(np.int16)
        ohm1 = np.zeros((ncols, 128, 5), dtype=BF16)
        ohv = np.eye(4, dtype=np.float32)[(node1 % 4)] * valid[:, :, None]
        ohm1[:, :, 0:4] = ohv.astype(BF16)
        ohm1[:, :, 4] = np.where(valid, 0.0, -1e30).astype(BF16)

        g = gpos_of_node[node1]        # permuted global position (L2 table)
        idx2_flat = np.where(valid, g // 4, 0).astype(np.int16)
        ohm2 = np.zeros((ncols, 128, 5), dtype=BF16)
        ohv2 = np.eye(4, dtype=np.float32)[(g % 4)] * valid[:, :, None]
        ohm2[:, :, 0:4] = ohv2.astype(BF16)
        ohm2[:, :, 4] = ohm1[:, :, 4]

        # wrap indices per gather part (layer-specific part boundaries)
        w1l = [_wrap_idx(idx1_flat[gb : gb + kt].reshape(-1))
               for (_t, gb, kt, _f, _l) in parts1]
        w2l = [_wrap_idx(idx2_flat[gb : gb + kt].reshape(-1))
               for (_t, gb, kt, _f, _l) in parts2]
        idx1_w = np.concatenate(w1l, axis=1)
        idx2_w = np.concatenate(w2l, axis=1)

        per_core_inputs.append(dict(
            idx1=idx1_w,
            idx2=idx2_w,
            ohm1=np.ascontiguousarray(ohm1.transpose(1, 0, 2)),
            ohm2=np.ascontiguousarray(ohm2.transpose(1, 0, 2)),
        ))
    return per_core_inputs


_BUILD_CACHE = {}


def _build(K, col0, parts1, parts2, totc, ncols):
    import concourse.bass as bass
    import concourse.bacc as bacc
    import concourse.mybir as mybir
    import concourse.tile as tile
    from concourse.masks import make_identity

    f32 = mybir.dt.float32
    bf16 = mybir.dt.bfloat16
    i16 = mybir.dt.int16
    AX = mybir.AxisListType.X
    OP = mybir.AluOpType
    ACT = mybir.ActivationFunctionType

    nc = bacc.Bacc("TRN2", target_bir_lowering=False, debug=False,
                   num_devices=NCORES, num_swdge_queues=4)

    xTh = nc.dram_tensor("xTh", [IN, SH], bf16, kind="ExternalInput")
    xTl = nc.dram_tensor("xTl", [IN, SH], bf16, kind="ExternalInput")
    w1eh = nc.dram_tensor("w1eh", [IN, 80], bf16, kind="ExternalInput")
    w1el = nc.dram_tensor("w1el", [IN, 80], bf16, kind="ExternalInput")
    b1e = nc.dram_tensor("b1e", [128, 80], f32, kind="ExternalInput")
    w2e = nc.dram_tensor("w2e", [HID, 4], bf16, kind="ExternalInput")
    b2e = nc.dram_tensor("b2e", [128, 4], f32, kind="ExternalInput")
    idx1 = nc.dram_tensor("idx1", [128, totc], i16, kind="ExternalInput")
    idx2 = nc.dram_tensor("idx2", [128, totc], i16, kind="ExternalInput")
    ohm1 = nc.dram_tensor("ohm1", [128, ncols, 5], bf16, kind="ExternalInput")
    ohm2 = nc.dram_tensor("ohm2", [128, ncols, 5], bf16, kind="ExternalInput")

    t1shard = nc.dram_tensor("t1shard", [SHR, T1W], bf16, kind="Internal")
    table1 = nc.dram_tensor("table1", [T1ROWS, T1W], bf16, kind="Internal",
                            addr_space="Shared")
    t2shard = nc.dram_tensor("t2shard", [PT // 4, T2W], bf16, kind="Internal")
    table2 = nc.dram_tensor("table2", [T2ROWS, T2W], bf16, kind="Internal",
                            addr_space="Shared")
    outp = nc.dram_tensor("outp", [PT, OUT], f32, kind="ExternalOutput")

    with tile.TileContext(nc) as tc:
        with (
            tc.tile_pool(name="const", bufs=1) as cpool,
            tc.tile_pool(name="node", bufs=3) as npool,
            tc.tile_pool(name="npsum", bufs=2, space="PSUM") as npsum,
            tc.tile_pool(name="gth", bufs=2) as gpool,
            tc.tile_pool(name="edge", bufs=3) as epool,
            tc.tile_pool(name="accs", bufs=2) as apool,
            tc.tile_pool(name="fin", bufs=2) as fpool,
            tc.tile_pool(name="fpsum", bufs=2, space="PSUM") as fpsum,
        ):
            ident = cpool.tile([128, 128], bf16)
            make_identity(nc, ident[:])
            zero1 = cpool.tile([128, 1], f32)
            nc.vector.memset(zero1[:], 0.0)
            w1hs = cpool.tile([IN, 80], bf16)
            nc.sync.dma_start(out=w1hs[:], in_=w1eh[:])
            w1ls = cpool.tile([IN, 80], bf16)
            nc.sync.dma_start(out=w1ls[:], in_=w1el[:])
            b1es = cpool.tile([128, 80], f32)
            nc.sync.dma_start(out=b1es[:], in_=b1e[:])
            w2es = cpool.tile([HID, 4], bf16)
            nc.sync.dma_start(out=w2es[:], in_=w2e[:])
            b2es = cpool.tile([128, 4], f32)
            nc.sync.dma_start(out=b2es[:], in_=b2e[:])

            # ---- node phase (sharded 8x): t1shard rows = [al_src | h+b1 | al_dst]
            CH = 512
            chunks = [(i * CH, CH) for i in range(SH // CH)]
            if SH % CH:
                chunks.append((SH - SH % CH, SH % CH))
            for (off, sz) in chunks:
                nb = sz // 128
                xhs = npool.tile([128, CH], bf16, tag="xhs")
                nc.sync.dma_start(out=xhs[:, 0:sz], in_=xTh[:, off : off + sz])
                xls = npool.tile([128, CH], bf16, tag="xls")
                nc.sync.dma_start(out=xls[:, 0:sz], in_=xTl[:, off : off + sz])
                ps = npsum.tile([128, 4, 80], f32, tag="ps")
                for i in range(nb):
                    # bf16x3: h = xh@Wh + xl@Wh + xh@Wl (~f32 accuracy)
                    nc.tensor.matmul(out=ps[:, i, :],
                                     lhsT=xhs[:, i * 128 : (i + 1) * 128],
                                     rhs=w1hs[:], start=True, stop=False)
                    nc.tensor.matmul(out=ps[:, i, :],
                                     lhsT=xls[:, i * 128 : (i + 1) * 128],
                                     rhs=w1hs[:], start=False, stop=False)
                    nc.tensor.matmul(out=ps[:, i, :],
                                     lhsT=xhs[:, i * 128 : (i + 1) * 128],
                                     rhs=w1ls[:], start=False, stop=True)
                t1c = npool.tile([128, 4, 80], bf16, tag="t1c")
                nc.vector.tensor_tensor(
                    out=t1c[:, 0:nb, :], in0=ps[:, 0:nb, :],
                    in1=b1es[:].unsqueeze(1).to_broadcast([128, nb, 80]),
                    op=OP.add)
                for i in range(nb):
                    r0 = off // 4 + 32 * i
                    dst_ap = t1shard[r0 : r0 + 32, 0:320].rearrange(
                        "r (n v) -> r n v", v=80)
                    nc.scalar.dma_start(out=dst_ap, in_=t1c[:, i, :])

            nc.gpsimd.collective_compute(
                "AllGather",
                OP.bypass,
                replica_groups=[list(range(NCORES))],
                ins=[t1shard[:]],
                outs=[table1[:]],
            )

            # ---- edge phase runner
            def select4(out_ap, gt, kt, voff, nv, ohm_t, tag, ew, ktmax):
                # 4-way one-hot select as copy + 3 predicated overwrites.
                # TensorCopy runs at 2-4x on DVE; the broadcast-mask
                # mult/add formulation ran at 1x (stride-0 operands
                # disable the 2x_1p mode). Pad slots (all-zero one-hot)
                # keep sub-node 0's finite values; the -1e30 pad mask
                # zeroes their exp weight downstream.
                nc.vector.tensor_copy(out=out_ap,
                                      in_=gt[:, 0:kt, voff : voff + nv])
                for i in range(1, 4):
                    # CopyPredicated wants an int mask; bf16 1.0 = 0x3F80
                    nc.vector.copy_predicated(
                        out=out_ap,
                        mask=ohm_t[:, 0:kt, i : i + 1].bitcast(i16)
                            .to_broadcast([128, kt, nv]),
                        data=gt[:, 0:kt, i * ew + voff : i * ew + voff + nv])

            def edge_phase(layer):
                if layer == 1:
                    idxT, ohmT, tabT, EW, NV, EWN = idx1, ohm1, table1, T1W, 72, 80
                    parts, KTMAX, GCOLS = parts1, PARTC1, GCOLS1
                else:
                    idxT, ohmT, tabT, EW, NV, EWN = idx2, ohm2, table2, T2W, 4, 4
                    parts, KTMAX, GCOLS = parts2, PARTC2, GCOLS2
                NH = H1 if layer == 1 else 1
                NCCH = C1 if layer == 1 else OUT
                TROWS = NH + NH * NCCH   # exp rows + weighted-payload rows

                ioff = 0
                gq = 0
                nalt = 0
                adt = None
                acc = None
                for (t, gbase, kt, first, last) in parts:
                    eng = nc.sync if (nalt % 2 == 0) else nc.scalar
                    nalt += 1
                    gt = gpool.tile([128, KTMAX, EW], bf16, tag=f"gt{layer}")
                    idx_t = epool.tile([128, KTMAX * 8], i16, tag=f"ix{layer}")
                    eng.dma_start(out=idx_t[:, 0 : kt * 8],
                                  in_=idxT[:, ioff : ioff + kt * 8])
                    for c0 in range(0, kt, GCOLS):
                        ncc = min(GCOLS, kt - c0)
                        nc.gpsimd.dma_gather(
                            gt[:, c0 : c0 + ncc, :], tabT[:],
                            idx_t[:, c0 * 8 : (c0 + ncc) * 8],
                            ncc * 128, ncc * 128, EW, queue_num=gq % 4)
                        gq += 1
                    ioff += kt * 8
                    ohm_t = epool.tile([128, KTMAX, 5], bf16, tag=f"oh{layer}")
                    eng.dma_start(out=ohm_t[:, 0:kt, :],
                                  in_=ohmT[:, gbase : gbase + kt, :])

                    V = epool.tile([128, KTMAX, NV], bf16, tag=f"V{layer}")
                    select4(V[:, 0:kt, :], gt, kt, 0, NV, ohm_t,
                            f"v{layer}", EWN, KTMAX)
                    if first:
                        if layer == 1:
                            adt_t = epool.tile([128, 1, NH], bf16,
                                               tag=f"adt{layer}")
                            select4(adt_t[:], gt, 1, NV, NH, ohm_t,
                                    f"a{layer}", EWN, KTMAX)
                            adt = adt_t[:]
                        else:
                            # L2 row = [a2s.g, g0, g1, a2d.g]; col 0 is the
                            # dst's own row, so al_dst is V[:, 0, 3]
                            adt = V[:, 0:1, 3:4]

                    eT = epool.tile([128, KTMAX, NH], f32, tag=f"e{layer}")
                    nc.vector.tensor_tensor(
                        out=eT[:, 0:kt, :], in0=V[:, 0:kt, 0:NH],
                        in1=adt.to_broadcast([128, kt, NH]),
                        op=OP.add)
                    nc.vector.tensor_tensor(
                        out=eT[:, 0:kt, :], in0=eT[:, 0:kt, :],
                        in1=ohm_t[:, 0:kt, 4:5].to_broadcast([128, kt, NH]),
                        op=OP.add)
                    # exp(lrelu(e)) = max(exp(e), exp(0.2e))
                    x1 = epool.tile([128, KTMAX, NH], bf16, tag=f"x1{layer}")
                    nc.scalar.activation(out=x1[:, 0:kt, :], in_=eT[:, 0:kt, :],
                                         func=ACT.Exp)
                    x2 = epool.tile([128, KTMAX, NH], bf16, tag=f"x2{layer}")
                    nc.scalar.activation(out=x2[:, 0:kt, :], in_=eT[:, 0:kt, :],
                                         func=ACT.Exp, scale=NEG)
                    W = epool.tile([128, TROWS, KTMAX], bf16,
                                   tag=f"W{layer}")
                    nc.vector.tensor_tensor(
                        out=W[:, 0:NH, 0:kt].rearrange("p h c -> p c h"),
                        in0=x1[:, 0:kt, :], in1=x2[:, 0:kt, :], op=OP.max)
                    nc.vector.tensor_tensor(
                        out=W[:, NH : NH + NH * NCCH, 0:kt].rearrange(
                            "p (h c) j -> p h c j", h=NH),
                        in0=V[:, 0:kt, NH : NH + NH * NCCH].rearrange(
                            "p j (h c) -> p h c j", h=NH),
                        in1=W[:, 0:NH, 0:kt].unsqueeze(2).to_broadcast(
                            [128, NH, NCCH, kt]),
                        op=OP.mult)
                    if first:
                        acc = apool.tile([128, TROWS], f32,
                                         tag=f"acc{layer}")
                        nc.vector.tensor_reduce(
                            out=acc[:], in_=W[:, :, 0:kt], axis=AX,
                            op=OP.add)
                    else:
                        red = apool.tile([128, TROWS], f32,
                                         tag=f"red{layer}")
                        nc.vector.tensor_reduce(
                            out=red[:], in_=W[:, :, 0:kt], axis=AX,
                            op=OP.add)
                        nc.vector.tensor_tensor(out=acc[:], in0=acc[:],
                                                in1=red[:], op=OP.add)
                    if last:
                        finalize(layer, t, acc)

            def finalize(layer, t, a):
                NH = H1 if layer == 1 else 1
                NCCH = C1 if layer == 1 else OUT
                rden = fpool.tile([128, NH], f32, tag="rden")
                nc.vector.reciprocal(out=rden[:], in_=a[:, 0:NH])
                if layer == 1:
                    y = fpool.tile([128, HID], f32, tag="y")
                    nc.vector.tensor_tensor(
                        out=y[:].rearrange("p (h c) -> p h c", h=NH),
                        in0=a[:, NH : NH + HID].rearrange("p (h c) -> p h c",
                                                          h=NH),
                        in1=rden[:].unsqueeze(2).to_broadcast([128, NH, NCCH]),
                        op=OP.mult)
                    # elu(y)+1 = relu(y) + exp(min(y,0)); -1 folded into b2e
                    rp = fpool.tile([128, HID], f32, tag="rp")
                    nc.vector.tensor_tensor(
                        out=rp[:], in0=y[:],
                        in1=zero1[:, 0:1].to_broadcast([128, HID]), op=OP.max)
                    zm = fpool.tile([128, HID], f32, tag="zm")
                    nc.vector.tensor_tensor(out=zm[:], in0=y[:], in1=rp[:],
                                            op=OP.subtract)
                    e1 = fpool.tile([128, HID], f32, tag="e1")
                    nc.scalar.activation(out=e1[:], in_=zm[:], func=ACT.Exp)
                    zb = fpool.tile([128, HID], bf16, tag="zb")
                    nc.vector.tensor_tensor(out=zb[:], in0=rp[:], in1=e1[:],
                                            op=OP.add)
                    # G2 row = zs @ W2e + b2e'
                    zT_ps = fpsum.tile([HID, 128], bf16, tag="zTp")
                    nc.tensor.transpose(out=zT_ps[:], in_=zb[:],
                                        identity=ident[:])
                    zTs = fpool.tile([HID, 128], bf16, tag="zTs")
                    nc.vector.tensor_copy(out=zTs[:], in_=zT_ps[:])
                    g2_ps = fpsum.tile([128, 4], f32, tag="g2p")
                    nc.tensor.matmul(out=g2_ps[:], lhsT=zTs[:], rhs=w2es[:],
                                     start=True, stop=True)
                    g2s = fpool.tile([128, 4], bf16, tag="g2s")
                    nc.vector.tensor_tensor(out=g2s[:], in0=g2_ps[:],
                                            in1=b2es[:], op=OP.add)
                    dst_ap = t2shard[t * 32 : t * 32 + 32, 0:16].rearrange(
                        "r (n v) -> r n v", v=4)
                    nc.sync.dma_start(out=dst_ap, in_=g2s[:])
                else:
                    o = fpool.tile([128, OUT], f32, tag="o2")
                    nc.vector.tensor_tensor(
                        out=o[:], in0=a[:, 1 : 1 + OUT],
                        in1=rden[:].to_broadcast([128, OUT]),
                        op=OP.mult)
                    nc.sync.dma_start(
                        out=outp[t * 128 : (t + 1) * 128, :], in_=o[:])

            edge_phase(1)

            nc.gpsimd.collective_compute(
                "AllGather",
                OP.bypass,
                replica_groups=[list(range(NCORES))],
                ins=[t2shard[:]],
                outs=[table2[:]],
            )

            edge_phase(2)

    nc.compile()
    return nc


def kernel(**inputs):
    from concourse.bass_utils import run_bass_kernel_spmd

    x = np.asarray(inputs["x"], dtype=np.float32)
    ei = np.asarray(inputs["edge_index"]).astype(np.int64)
    w1 = np.asarray(inputs["W1"], dtype=np.float32)
    a1s = np.asarray(inputs["a1_src"], dtype=np.float32)
    a1d = np.asarray(inputs["a1_dst"], dtype=np.float32)
    b1 = np.asarray(inputs["b1"], dtype=np.float32)
    w2 = np.asarray(inputs["W2"], dtype=np.float32)
    a2s = np.asarray(inputs["a2_src"], dtype=np.float32)
    a2d = np.asarray(inputs["a2_dst"], dtype=np.float32)
    b2 = np.asarray(inputs["b2"], dtype=np.float32)

    src = ei[0]
    dst = ei[1]

    datas, K, col0, parts1, parts2, ncols = _plan(src, dst)
    totc = 8 * ncols

    # permuted global position of each node for the L2 table
    gpos_of_node = np.zeros(NPAD, dtype=np.int64)
    for c in range(NCORES):
        perm = datas[c]["perm"]  # [PT] local dst ids (or -1)
        real = perm >= 0
        gpos_of_node[c * ND + perm[real]] = c * PT + np.nonzero(real)[0]

    per_core = _pack_inputs(datas, gpos_of_node, parts1, parts2, ncols)

    # weights
    A1s = np.zeros((HID, H1), dtype=np.float32)
    A1d = np.zeros((HID, H1), dtype=np.float32)
    for h in range(H1):
        A1s[h * C1 : (h + 1) * C1, h] = a1s[h]
        A1d[h * C1 : (h + 1) * C1, h] = a1d[h]
    w1e = np.concatenate([w1 @ A1s, w1, w1 @ A1d], axis=1)      # [128, 80]
    w2e = np.concatenate([w2 @ a2s.T, w2, w2 @ a2d.T], axis=1)  # [64, 4]
    b1e = np.zeros((128, 80), dtype=np.float32)
    b1e[:, 8 : 8 + HID] = b1[None, :]
    # b2 row with the elu -1 shift folded in: t2row = zs@w2e + (b2row - 1@w2e)
    b2row = np.zeros(4, dtype=np.float32)
    b2row[1 : 1 + OUT] = b2
    b2adj = b2row - np.ones(HID, dtype=np.float32) @ w2e
    b2e = np.tile(b2adj[None, :], (128, 1)).astype(np.float32)

    xp = np.zeros((NPAD, IN), dtype=np.float32)
    xp[:N] = x
    xT_f32 = xp.T                                      # [128, NPAD]
    xTh_all = xT_f32.astype(BF16)
    xTl_all = (xT_f32 - xTh_all.astype(np.float32)).astype(BF16)
    w1h = w1e.astype(BF16)
    w1l = (w1e - w1h.astype(np.float32)).astype(BF16)

    key = (PARTC1, PARTC2, tuple(K.tolist()))
    if key not in _BUILD_CACHE:
        _BUILD_CACHE[key] = _build(K, col0, parts1, parts2, totc, ncols)
    nc = _BUILD_CACHE[key]

    common = dict(w1eh=w1h, w1el=w1l, b1e=b1e, w2e=w2e.astype(BF16),
                  b2e=b2e)
    in_maps = []
    for c in range(NCORES):
        m = dict(common)
        m.update(per_core[c])
        m["xTh"] = np.ascontiguousarray(xTh_all[:, c * SH : (c + 1) * SH])
        m["xTl"] = np.ascontiguousarray(xTl_all[:, c * SH : (c + 1) * SH])
        in_maps.append(m)

    global _LAST_IN_MAPS
    _LAST_IN_MAPS = in_maps
    res = run_bass_kernel_spmd(nc, in_maps, list(range(NCORES)))

    out = np.zeros((N, OUT), dtype=np.float32)
    for c in range(NCORES):
        op = res.results[c]["outp"]       # [PT, 2] in permuted order
        perm = datas[c]["perm"]
        real = perm >= 0
        out[c * ND + perm[real]] = op[real]
    return out
